# revision 1
# baseline (speedup 1.0000x reference)
"""Masked 3D patch embedding kernel for Trainium2 (8 NeuronCores, SPMD).

Reference computation (see problem):
  - im2col 4x4x4 patches of x [4,96,96,96,1] -> [4, 13824, 64]
  - unmasked tokens (perm[10368:]): gather -> @proj_w -> +proj_b +penc
  - masked tokens  (perm[:10368]): mask_token @ proj_w -> +proj_b +penc
    (batch-invariant -> computed once, replicated on host assembly)
  - concat, LayerNorm(eps=1e-3) * gamma + beta
  - returns (out [4,24,24,24,768], mask_idx [10368] int32)

Device strategy (per core, SPMD over 8 cores):
  - indirect-DMA gather of this core's unmasked patch rows (the embedding
    lookup) from the im2col'd table in DRAM
  - PE transpose -> stationary patches [64,128], matmul vs proj_w [64,768]
  - pos-enc (+bias) added in PSUM via identity-matmul accumulation
  - LayerNorm: bn_stats/bn_aggr (DVE) + Sqrt/reciprocal, normalize on ACT
    straight out of PSUM, results streamed to DRAM via HWDGE
Work split: unmasked 3456 j's -> 432/core (x4 batches); masked 10368 -> 1296/core.
"""

import sys

if "/opt/trn_rl_repo" not in sys.path:
    sys.path.insert(0, "/opt/trn_rl_repo")

import numpy as np

# ---------------- problem constants (hardcoded) ----------------
B = 4
HWD = 96
P0 = 4
E = 768
nH = 24
T = nH * nH * nH          # 13824 tokens
M = int(T * 0.75)         # 10368 masked
U = T - M                 # 3456 unmasked
K = P0 * P0 * P0          # 64 patch dim
EPS = 1e-3

N_CORES = 8
U_C = U // N_CORES        # 432 unmasked output tokens per core
U_TILES = 4               # padded to 512 = 4 x 128
M_C = M // N_CORES        # 1296 masked tokens per core
M_TILES = 11              # 10 x 128 + 16
M_PAD = M_TILES * 128     # 1408


def _penc_np():
    """numpy mirror of reference _sin_pos_enc_3d -> [T, E] float32."""
    channels = int(np.ceil(E / 6) * 2)
    if channels % 2:
        channels += 1
    inv_freq = (1.0 / np.power(10000, np.arange(0, channels, 2) / np.float32(channels))).astype(np.float32)

    def emb1(n):
        s = np.arange(n, dtype=np.float32)[:, None] * inv_freq[None, :]
        return np.stack([np.sin(s), np.cos(s)], axis=-1).reshape(n, channels)

    ex = np.broadcast_to(emb1(nH)[:, None, None, :], (nH, nH, nH, channels))
    ey = np.broadcast_to(emb1(nH)[None, :, None, :], (nH, nH, nH, channels))
    ez = np.broadcast_to(emb1(nH)[None, None, :, :], (nH, nH, nH, channels))
    emb = np.concatenate([ex, ey, ez], axis=-1)[..., :E]
    return np.ascontiguousarray(emb.reshape(T, E)).astype(np.float32)


# ---------------- Bass program (built once, cached) ----------------
_PROG_CACHE = {}


def _build_program(affine: bool):
    """Build the per-core Bass/Tile program. `affine`: apply gamma/beta."""
    import concourse.bass as bass
    import concourse.bacc as bacc
    import concourse.tile as tile
    from concourse import mybir
    from concourse.masks import make_identity

    f32 = mybir.dt.float32

    nc = bacc.Bacc("TRN2", target_bir_lowering=False, debug=False,
                   enable_asserts=True, num_devices=N_CORES)

    pf_d = nc.dram_tensor("pf", [B * T, K], f32, kind="ExternalInput")
    idx_d = nc.dram_tensor("idxs", [128, B * U_TILES], mybir.dt.int32, kind="ExternalInput")
    pencu_d = nc.dram_tensor("penc_u", [U_TILES * 128, E], f32, kind="ExternalInput")
    mtokT_d = nc.dram_tensor("mtokT", [K, M_PAD], f32, kind="ExternalInput")
    pencm_d = nc.dram_tensor("penc_m", [M_PAD, E], f32, kind="ExternalInput")
    w_d = nc.dram_tensor("w", [K, E], f32, kind="ExternalInput")
    if affine:
        gb_d = nc.dram_tensor("gb", [2, E], f32, kind="ExternalInput")

    outu_d = nc.dram_tensor("out_u", [B, U_C, E], f32, kind="ExternalOutput")
    outm_d = nc.dram_tensor("out_m", [M_C, E], f32, kind="ExternalOutput")

    with tile.TileContext(nc) as tc:
        with (
            tc.tile_pool(name="singles", bufs=1) as singles,
            tc.tile_pool(name="gat", bufs=6) as gat_pool,
            tc.tile_pool(name="lhs", bufs=4) as lhs_pool,
            tc.tile_pool(name="pencm", bufs=4) as pencm_pool,
            tc.tile_pool(name="outs", bufs=6) as out_pool,
            tc.tile_pool(name="stats", bufs=8) as st_pool,
            tc.tile_pool(name="ypsum", bufs=2, space="PSUM") as ypsum_pool,
            tc.tile_pool(name="tpsum", bufs=2, space="PSUM") as tpsum_pool,
        ):
            # ---- resident constants ----
            w_sb = singles.tile([K, E], f32, tag="w")
            nc.sync.dma_start(out=w_sb[:], in_=w_d.ap())
            ident = singles.tile([128, 128], f32, tag="ident")
            make_identity(nc, ident[:])
            eps_sb = singles.tile([128, 1], f32, tag="eps")
            nc.vector.memset(eps_sb[:], EPS)
            idx_sb = singles.tile([128, B * U_TILES], mybir.dt.int32, tag="idx")
            nc.sync.dma_start(out=idx_sb[:], in_=idx_d.ap())
            mtokT_sb = singles.tile([K, M_PAD], f32, tag="mtokT")
            nc.sync.dma_start(out=mtokT_sb[:], in_=mtokT_d.ap())
            pencu_sb = []
            for g in range(U_TILES):
                t = singles.tile([128, E], f32, tag=f"pencu{g}")
                nc.sync.dma_start(out=t[:], in_=pencu_d.ap()[g * 128:(g + 1) * 128, :])
                pencu_sb.append(t)
            if affine:
                gamma_sb = singles.tile([128, E], f32, tag="gamma")
                beta_sb = singles.tile([128, E], f32, tag="beta")
                gb_ap = gb_d.ap()
                nc.gpsimd.dma_start(
                    out=gamma_sb[:],
                    in_=bass.AP(tensor=gb_ap.tensor, offset=gb_ap.offset,
                                ap=[[0, 128], [1, E]]))
                nc.gpsimd.dma_start(
                    out=beta_sb[:],
                    in_=bass.AP(tensor=gb_ap.tensor, offset=gb_ap.offset + E,
                                ap=[[0, 128], [1, E]]))

            def project_ln_store(lhsT_ap, penc_ap, store_ap, cnt):
                """y = lhsT.T @ W + penc ; LN ; store first `cnt` rows."""
                y = ypsum_pool.tile([128, E], f32, tag="y", space="PSUM")
                nc.tensor.matmul(y[:, 0:512], lhsT=lhsT_ap, rhs=w_sb[:, 0:512],
                                 start=True, stop=False)
                nc.tensor.matmul(y[:, 512:E], lhsT=lhsT_ap, rhs=w_sb[:, 512:E],
                                 start=True, stop=False)
                nc.tensor.matmul(y[:, 0:512], lhsT=ident[:], rhs=penc_ap[:, 0:512],
                                 start=False, stop=True)
                nc.tensor.matmul(y[:, 512:E], lhsT=ident[:], rhs=penc_ap[:, 512:E],
                                 start=False, stop=True)
                # LayerNorm stats over E (3 x 256 subgroups)
                st = st_pool.tile([128, 3, 6], f32, tag="st")
                for s in range(3):
                    nc.vector.bn_stats(out=st[:, s, :], in_=y[:, s * 256:(s + 1) * 256])
                mv = st_pool.tile([128, 2], f32, tag="mv")
                nc.vector.bn_aggr(out=mv[:], in_=st[:])
                std = st_pool.tile([128, 1], f32, tag="std")
                nc.scalar.activation(out=std[:], in_=mv[:, 1:2],
                                     func=mybir.ActivationFunctionType.Sqrt,
                                     bias=eps_sb[:], scale=1.0)
                rstd = st_pool.tile([128, 1], f32, tag="rstd")
                nc.vector.reciprocal(out=rstd[:], in_=std[:])
                nbias = st_pool.tile([128, 1], f32, tag="nbias")
                nc.vector.tensor_scalar_mul(out=nbias[:], in0=mv[:, 0:1], scalar1=rstd[:])
                nc.scalar.mul(out=nbias[:], in_=nbias[:], mul=-1.0)
                o = out_pool.tile([128, E], f32, tag="o")
                nc.scalar.activation(out=o[:], in_=y[:],
                                     func=mybir.ActivationFunctionType.Identity,
                                     bias=nbias[:], scale=rstd[:])
                if affine:
                    nc.vector.tensor_mul(out=o[:], in0=o[:], in1=gamma_sb[:])
                    nc.vector.tensor_add(out=o[:], in0=o[:], in1=beta_sb[:])
                nc.sync.dma_start(out=store_ap, in_=o[:cnt, :])

            # ---- unmasked tokens ----
            for b in range(B):
                for g in range(U_TILES):
                    q = b * U_TILES + g
                    cnt = min(128, U_C - g * 128)
                    gat = gat_pool.tile([128, K], f32, tag="gat")
                    nc.gpsimd.indirect_dma_start(
                        out=gat[:], out_offset=None, in_=pf_d.ap(),
                        in_offset=bass.IndirectOffsetOnAxis(ap=idx_sb[:, q:q + 1], axis=0),
                    )
                    tp = tpsum_pool.tile([K, 128], f32, tag="tp", space="PSUM")
                    nc.tensor.transpose(out=tp[:], in_=gat[:], identity=ident[:])
                    lt = lhs_pool.tile([K, 128], f32, tag="lt")
                    nc.vector.tensor_copy(out=lt[:], in_=tp[:])
                    project_ln_store(
                        lt[:], pencu_sb[g][:],
                        outu_d.ap()[b, g * 128:g * 128 + cnt, :], cnt)

            # ---- masked tokens (batch-invariant, computed once) ----
            for g in range(M_TILES):
                cnt = min(128, M_C - g * 128)
                pm = pencm_pool.tile([128, E], f32, tag="pm")
                nc.sync.dma_start(out=pm[:], in_=pencm_d.ap()[g * 128:(g + 1) * 128, :])
                project_ln_store(
                    mtokT_sb[:, g * 128:(g + 1) * 128], pm[:],
                    outm_d.ap()[g * 128:g * 128 + cnt, :], cnt)

    nc.compile()
    return nc


def _get_program(affine: bool):
    if affine not in _PROG_CACHE:
        _PROG_CACHE[affine] = _build_program(affine)
    return _PROG_CACHE[affine]


def _prepare_core_inputs(pf, pencb, mask_token, proj_w, unmask_idx, gamma, beta, affine):
    """Build the 8 per-core input dicts (host-side layout prep only)."""
    mtok = np.asarray(mask_token[0], dtype=np.float32)      # [M, K]
    in_maps = []
    for c in range(N_CORES):
        # unmasked gather indices: global pf row = b*T + unmask_idx[j]
        idxs = np.zeros((128, B * U_TILES), np.int32)
        for b_ in range(B):
            for g in range(U_TILES):
                j0 = c * U_C + g * 128
                cnt = min(128, U_C - g * 128)
                if cnt > 0:
                    idxs[:cnt, b_ * U_TILES + g] = b_ * T + unmask_idx[j0:j0 + cnt]
        penc_u = np.zeros((U_TILES * 128, E), np.float32)
        penc_u[:U_C] = pencb[c * U_C:(c + 1) * U_C]
        penc_m = np.zeros((M_PAD, E), np.float32)
        penc_m[:M_C] = pencb[U + c * M_C:U + (c + 1) * M_C]
        mtokT = np.zeros((K, M_PAD), np.float32)
        mtokT[:, :M_C] = mtok[c * M_C:(c + 1) * M_C].T
        m = {
            "pf": pf,
            "idxs": idxs,
            "penc_u": penc_u,
            "mtokT": mtokT,
            "penc_m": penc_m,
            "w": np.ascontiguousarray(proj_w, dtype=np.float32),
        }
        if affine:
            m["gb"] = np.stack([gamma, beta]).astype(np.float32)
        in_maps.append(m)
    return in_maps


def kernel(**inputs):
    from concourse import bass_utils

    x = np.asarray(inputs["x"], dtype=np.float32)
    proj_w = np.asarray(inputs["proj_w"], dtype=np.float32)
    proj_b = np.asarray(inputs["proj_b"], dtype=np.float32)
    mask_token = np.asarray(inputs["mask_token"], dtype=np.float32)
    gamma = np.asarray(inputs["gamma"], dtype=np.float32)
    beta = np.asarray(inputs["beta"], dtype=np.float32)
    perm = np.asarray(inputs["perm"], dtype=np.int32)

    mask_idx = perm[:M]
    unmask_idx = perm[M:]
    affine = not (np.all(gamma == 1.0) and np.all(beta == 0.0))

    # host layout prep: im2col + output-ordered (pos-enc + bias) table
    pf = x.reshape(B, nH, P0, nH, P0, nH, P0).transpose(0, 1, 3, 5, 2, 4, 6)
    pf = np.ascontiguousarray(pf).reshape(B * T, K)
    penc = _penc_np()
    pencb = np.empty((T, E), np.float32)
    pencb[:U] = penc[unmask_idx] + proj_b
    pencb[U:] = penc[mask_idx] + proj_b

    nc = _get_program(affine)
    in_maps = _prepare_core_inputs(pf, pencb, mask_token, proj_w, unmask_idx,
                                   gamma, beta, affine)
    res = bass_utils.run_bass_kernel_spmd(nc, in_maps, core_ids=list(range(N_CORES)))

    out_tokens = np.empty((B, T, E), np.float32)
    for c in range(N_CORES):
        out_tokens[:, c * U_C:(c + 1) * U_C] = res.results[c]["out_u"]
        out_tokens[:, U + c * M_C:U + (c + 1) * M_C] = res.results[c]["out_m"][None]
    out = out_tokens.reshape(B, nH, nH, nH, E)
    return out, mask_idx


# revision 49
# speedup vs baseline: 2.3964x; 2.3964x over previous
"""Masked 3D patch embedding kernel for Trainium2 (8 NeuronCores, SPMD).

Reference computation (see problem):
  - im2col 4x4x4 patches of x [4,96,96,96,1] -> [4, 13824, 64]
  - unmasked tokens (perm[10368:]): gather -> @proj_w -> +proj_b +penc
  - masked tokens  (perm[:10368]): mask_token @ proj_w -> +proj_b +penc
    (batch-invariant -> computed once, replicated on host assembly)
  - concat, LayerNorm(eps=1e-3) * gamma + beta
  - returns (out [4,24,24,24,768], mask_idx [10368] int32)

Device strategy (per core, SPMD over 8 cores):
  - transposing dma_gather of this core's unmasked patch rows (the
    embedding lookup) straight into PE-stationary layout [K, tokens]
  - fp16 operands on the PE (1 cycle/row vs fp32's 4), fp32 PSUM accumulate
  - pos-enc (+bias) added in PSUM via identity-matmul accumulation
  - LayerNorm: bn_stats/bn_aggr (DVE) + Sqrt/reciprocal, normalize on ACT
    straight out of PSUM, fp32 results streamed to DRAM via HWDGE
Work split: unmasked 3456 j's -> 432/core (x4 batches); masked 10368 -> 1296/core.
"""

import sys

if "/opt/trn_rl_repo" not in sys.path:
    sys.path.insert(0, "/opt/trn_rl_repo")

import numpy as np

# ---------------- problem constants (hardcoded) ----------------
B = 4
HWD = 96
P0 = 4
E = 768
nH = 24
T = nH * nH * nH          # 13824 tokens
M = int(T * 0.75)         # 10368 masked
U = T - M                 # 3456 unmasked
K = P0 * P0 * P0          # 64 patch dim
KP = 128                  # K zero-padded for the PE / 256B gather rows
EPS = 1e-3

N_CORES = 8
U_C = U // N_CORES        # 432 unmasked output tokens per core
UP_PAD = 896              # batch-pair pack: 2x432=864 valid, padded to 7x128
UP_TILES = 7
M_C = M // N_CORES        # 1296 masked tokens per core
M_TILES = 11              # 10 x 128 + 16
M_PAD = M_TILES * 128     # 1408


def _emb1_np(n=nH):
    """per-axis sin/cos table emb1(n) -> [n, 256] float32 (channels=256)."""
    channels = int(np.ceil(E / 6) * 2)
    if channels % 2:
        channels += 1
    inv_freq = (1.0 / np.power(10000, np.arange(0, channels, 2) / np.float32(channels))).astype(np.float32)
    s = np.arange(n, dtype=np.float32)[:, None] * inv_freq[None, :]
    return np.stack([np.sin(s), np.cos(s)], axis=-1).reshape(n, channels).astype(np.float32)


def _penc_np():
    """numpy mirror of reference _sin_pos_enc_3d -> [T, E] float32."""
    channels = int(np.ceil(E / 6) * 2)
    if channels % 2:
        channels += 1
    inv_freq = (1.0 / np.power(10000, np.arange(0, channels, 2) / np.float32(channels))).astype(np.float32)

    def emb1(n):
        s = np.arange(n, dtype=np.float32)[:, None] * inv_freq[None, :]
        return np.stack([np.sin(s), np.cos(s)], axis=-1).reshape(n, channels)

    ex = np.broadcast_to(emb1(nH)[:, None, None, :], (nH, nH, nH, channels))
    ey = np.broadcast_to(emb1(nH)[None, :, None, :], (nH, nH, nH, channels))
    ez = np.broadcast_to(emb1(nH)[None, None, :, :], (nH, nH, nH, channels))
    emb = np.concatenate([ex, ey, ez], axis=-1)[..., :E]
    return np.ascontiguousarray(emb.reshape(T, E)).astype(np.float32)


# ---------------- Bass program (built once, cached) ----------------
_PROG_CACHE = {}


def _build_program(affine: bool):
    """Build the per-core Bass/Tile program. `affine`: apply gamma/beta."""
    import concourse.bass as bass
    import concourse.bacc as bacc
    import concourse.tile as tile
    from concourse import mybir
    from concourse.masks import make_identity

    f32 = mybir.dt.float32
    f16 = mybir.dt.float16
    E4 = E - 512   # D-axis pos-enc chunk (cols 512:768)

    nc = bacc.Bacc("TRN2", target_bir_lowering=False, debug=False,
                   enable_asserts=True, num_devices=N_CORES)

    pf_d = nc.dram_tensor("pf", [B, T, KP], f16, kind="ExternalInput")
    idx_d = nc.dram_tensor("idxs", [128, 2 * (UP_PAD // 16)], mybir.dt.int16,
                           kind="ExternalInput")
    pencu_d = nc.dram_tensor("penc_u", [UP_PAD, E4], f16, kind="ExternalInput")
    mtokT_d = nc.dram_tensor("mtokT", [KP, M_PAD], f16, kind="ExternalInput")
    pencm_d = nc.dram_tensor("penc_m", [M_PAD, E4], f16, kind="ExternalInput")
    w_d = nc.dram_tensor("w", [KP, E], f16, kind="ExternalInput")
    if affine:
        gb_d = nc.dram_tensor("gb", [2, E], f32, kind="ExternalInput")

    outu_d = nc.dram_tensor("out_u", [B, U_C, E], f16, kind="ExternalOutput")
    outm_d = nc.dram_tensor("out_m", [M_C, E], f16, kind="ExternalOutput")

    IW = UP_PAD // 16  # idx columns per batch-pair (wrapped in 16 partitions)

    with tile.TileContext(nc) as tc:
        with (
            tc.tile_pool(name="singles", bufs=1) as singles,
            tc.tile_pool(name="stats", bufs=24) as st_pool,
            tc.tile_pool(name="ypsum", bufs=4, space="PSUM") as ypsum_pool,
        ):
            # ---- resident constants; critical-path loads on the SP ring
            # (the ACT ring is blocked early by LoadActFuncSet) ----
            w_sb = singles.tile([KP, E], f16, tag="w")
            nc.sync.dma_start(out=w_sb[:], in_=w_d.ap())
            mtokT_sb = singles.tile([KP, M_PAD], f16, tag="mtokT")
            nc.sync.dma_start(out=mtokT_sb[:, 0:256], in_=mtokT_d.ap()[:, 0:256])
            pm_sb = singles.tile([128, M_TILES, E - 512], f16, tag="pm")
            pencm_r = pencm_d.ap().rearrange("(g p) e -> p g e", p=128)
            for g in range(2):
                nc.sync.dma_start(out=pm_sb[:, g, :], in_=pencm_r[:, g, :])
            nc.sync.dma_start(out=mtokT_sb[:, 256:M_PAD], in_=mtokT_d.ap()[:, 256:M_PAD])
            ident = singles.tile([128, 128], f16, tag="ident")
            make_identity(nc, ident[:])
            # warm the ACT function table immediately: the LoadActFuncSet for
            # Sqrt/Copy/Identity must not land mid-pipeline behind DMA configs
            warm = singles.tile([1, 1], f32, tag="warm")
            nc.vector.memset(warm[:], 1.0)
            nc.scalar.activation(out=warm[:], in_=warm[:],
                                 func=mybir.ActivationFunctionType.Sqrt)
            nc.scalar.activation(out=warm[:], in_=warm[:],
                                 func=mybir.ActivationFunctionType.Copy)
            nc.scalar.activation(out=warm[:], in_=warm[:],
                                 func=mybir.ActivationFunctionType.Identity)
            idx_sb = singles.tile([128, 2 * IW], mybir.dt.int16, tag="idx")
            nc.sync.dma_start(out=idx_sb[:], in_=idx_d.ap())
            pencu_sb = singles.tile([128, UP_TILES, E - 512], f16, tag="pencu")
            nc.sync.dma_start(
                out=pencu_sb[:],
                in_=pencu_d.ap().rearrange("(g p) e -> p g e", p=128))
            if affine:
                gamma_sb = singles.tile([128, E], f16, tag="gamma")
                beta_sb = singles.tile([128, E], f16, tag="beta")
                gb_ap = gb_d.ap()
                nc.gpsimd.dma_start(
                    out=gamma_sb[:],
                    in_=bass.AP(tensor=gb_ap.tensor, offset=gb_ap.offset,
                                ap=[[0, 128], [1, E]]))
                nc.gpsimd.dma_start(
                    out=beta_sb[:],
                    in_=bass.AP(tensor=gb_ap.tensor, offset=gb_ap.offset + E,
                                ap=[[0, 128], [1, E]]))

            # ---- the embedding lookup: one packed transposing gather per
            # batch-PAIR (2*13824 rows < int16 max) -> [K=128, 896 tokens] fp16
            # in PE-stationary layout, 864 valid rows packed contiguously ----
            pf_flat = pf_d.ap().rearrange("b t k -> (b t) k")
            gathT = []
            for pr in range(2):
                gt = singles.tile([128, 1, UP_PAD], f16, tag=f"gathT{pr}")
                nc.gpsimd.dma_gather(
                    gt[:], pf_flat[pr * 2 * T:(pr + 1) * 2 * T],
                    idx_sb[:, pr * IW:(pr + 1) * IW],
                    num_idxs=UP_PAD, num_idxs_reg=UP_PAD,
                    elem_size=KP, transpose=True,
                )
                gathT.append(gt)

            # remaining masked pos-enc slices (SP ring; keep ACT free for LN ops)
            for g in range(2, M_TILES):
                nc.sync.dma_start(out=pm_sb[:, g, :], in_=pencm_r[:, g, :])

            def project_ln(lhsT_ap, penc_ap, o_ap, norm_on_dve=False):
                """o = LayerNorm(lhsT.T @ W + penc) into SBUF slice o_ap."""
                y = ypsum_pool.tile([128, E], f32, tag="y", space="PSUM")
                # cols 0:512: proj + H/W pos-enc + bias all in one matmul --
                # the lhsT pad rows carry onehot(h), onehot(w), and a ones slot,
                # matched by emb tables / proj_b rows in w_sb
                nc.tensor.matmul(y[:, 0:512], lhsT=lhsT_ap, rhs=w_sb[:, 0:512],
                                 start=True, stop=True)
                nc.tensor.matmul(y[:, 512:E], lhsT=lhsT_ap, rhs=w_sb[:, 512:E],
                                 start=True, stop=False)
                nc.tensor.matmul(y[:, 512:E], lhsT=ident[:], rhs=penc_ap,
                                 start=False, stop=True)
                # LayerNorm stats over E (2 x 384: equal groups keep bn_aggr exact)
                st = st_pool.tile([128, 2, 6], f32, tag="st")
                for s in range(2):
                    nc.vector.bn_stats(out=st[:, s:s + 1, :],
                                       in_=y[:, s * 384:(s + 1) * 384])
                mv = st_pool.tile([128, 2], f32, tag="mv")
                nc.vector.bn_aggr(out=mv[:], in_=st[:])
                rv = st_pool.tile([128, 1], f32, tag="rv")
                nc.vector.tensor_scalar(out=rv[:], in0=mv[:, 1:2],
                                        scalar1=EPS, scalar2=None,
                                        op0=mybir.AluOpType.add)
                nc.vector.reciprocal(out=rv[:], in_=rv[:])
                rstd = st_pool.tile([128, 1], f32, tag="rstd")
                nc.scalar.activation(out=rstd[:], in_=rv[:],
                                     func=mybir.ActivationFunctionType.Sqrt)
                if norm_on_dve:
                    # drain-tail tiles: normalize on the otherwise-idle DVE
                    nc.vector.tensor_scalar(out=o_ap, in0=y[:],
                                            scalar1=mv[:, 0:1], scalar2=rstd[:],
                                            op0=mybir.AluOpType.subtract,
                                            op1=mybir.AluOpType.mult)
                else:
                    nm = st_pool.tile([128, 1], f32, tag="nm")
                    nc.vector.tensor_scalar(out=nm[:], in0=mv[:, 0:1],
                                            scalar1=-1.0, scalar2=None,
                                            op0=mybir.AluOpType.mult)
                    nbias = st_pool.tile([128, 1], f32, tag="nbias")
                    nc.scalar.activation(out=nbias[:], in_=rstd[:],
                                         func=mybir.ActivationFunctionType.Copy,
                                         bias=0.0, scale=nm[:])
                    nc.scalar.activation(out=o_ap, in_=y[:],
                                         func=mybir.ActivationFunctionType.Identity,
                                         bias=nbias[:], scale=rstd[:])
                if affine:
                    nc.vector.tensor_mul(out=o_ap, in0=o_ap, in1=gamma_sb[:])
                    nc.vector.tensor_add(out=o_ap, in0=o_ap, in1=beta_sb[:])

            # ---- masked tokens first (batch-invariant, computed once);
            # the unmasked gathers overlap this phase ----
            om = singles.tile([128, M_TILES, E], f16, tag="outm")
            for g in range(M_TILES):
                project_ln(mtokT_sb[:, g * 128:(g + 1) * 128],
                           pm_sb[:, g, :], om[:, g, :])
                # chunked stores as groups complete (pipelined drain)
                if g in (3, 7):
                    g0 = g - 3
                    nc.sync.dma_start(
                        out=outm_d.ap()[g0 * 128:(g + 1) * 128, :].rearrange(
                            "(a p) e -> p a e", p=128),
                        in_=om[:, g0:g + 1, :])
            mfull = M_C // 128                       # 10 full tiles
            nc.sync.dma_start(
                out=outm_d.ap()[8 * 128:mfull * 128, :].rearrange(
                    "(a p) e -> p a e", p=128),
                in_=om[:, 8:mfull, :])
            mrem = M_C - mfull * 128                 # 16-row tail
            nc.sync.dma_start(
                out=outm_d.ap()[mfull * 128:M_C, :],
                in_=om[:mrem, mfull, :])

            # ---- unmasked tokens: 7 packed tiles per batch-pair; packed row
            # r maps to (b = pr*2 + r//432, j = r%432) ----
            for pr in range(2):
                b0, b1 = 2 * pr, 2 * pr + 1
                ob = singles.tile([128, UP_TILES, E], f16, tag=f"outu{pr}")
                for g in range(UP_TILES):
                    project_ln(gathT[pr][:, 0, g * 128:(g + 1) * 128],
                               pencu_sb[:, g, :], ob[:, g, :],
                               norm_on_dve=False)
                    if g == 1:
                        nc.sync.dma_start(
                            out=outu_d.ap()[b0, 0:256, :].rearrange(
                                "(a p) e -> p a e", p=128),
                            in_=ob[:, 0:2, :])
                    if g == 5:
                        nc.sync.dma_start(
                            out=outu_d.ap()[b1, 80:336, :].rearrange(
                                "(a p) e -> p a e", p=128),
                            in_=ob[:, 4:6, :])
                # b0 rows 256:432 (tile 2 + tile 3 partitions 0:48)
                nc.sync.dma_start(out=outu_d.ap()[b0, 256:384, :], in_=ob[:, 2, :])
                nc.sync.dma_start(out=outu_d.ap()[b0, 384:U_C, :], in_=ob[:48, 3, :])
                # b1 rows: tile 3 partitions 48:128 -> j 0:80; tile 6 p 0:96 -> j 336:432
                nc.sync.dma_start(out=outu_d.ap()[b1, 0:80, :], in_=ob[48:128, 3, :])
                nc.sync.dma_start(out=outu_d.ap()[b1, 336:U_C, :], in_=ob[:96, 6, :])

    nc.compile()
    return nc


def _get_program(affine: bool):
    if affine not in _PROG_CACHE:
        _PROG_CACHE[affine] = _build_program(affine)
    return _PROG_CACHE[affine]


def _wrap_idxs(idx_flat):
    """[U_PAD] int -> [128, U_PAD//16] int16: idx i at [i % 16, i // 16],
    rows replicated to all 128 partitions (16-partition pattern x 8 cores)."""
    w = np.asarray(idx_flat, np.int16).reshape(-1, 16).T        # [16, IW]
    return np.tile(w, (8, 1))                                    # [128, IW]


def _prepare_core_inputs(pf, pencb16, mask_token, proj_w, proj_b, mask_idx,
                         unmask_idx, gamma, beta, affine):
    """Build the 8 per-core input dicts (host-side layout prep only).

    lhsT pad-row encoding (rows 64:128 of pf / mtokT):
      64:88  onehot(h)   -> w rows 64:88  = embH table in cols 0:256
      88:112 onehot(w)   -> w rows 88:112 = embW table in cols 256:512
      112    ones        -> w row 112     = proj_b[0:512] in cols 0:512
    so the cols-0:512 matmul includes the H/W pos-enc + bias directly."""
    emb = _emb1_np()                                     # [24, 256]
    mtokT_full = np.zeros((KP, M), np.float16)
    mtokT_full[:K] = np.asarray(mask_token[0], np.float32).T.astype(np.float16)
    mh, mw = mask_idx // (nH * nH), (mask_idx // nH) % nH
    mtokT_full[64 + mh, np.arange(M)] = 1.0
    mtokT_full[88 + mw, np.arange(M)] = 1.0
    mtokT_full[112, :] = 1.0
    w16 = np.zeros((KP, E), np.float16)
    w16[:K] = np.asarray(proj_w, np.float32).astype(np.float16)
    w16[64:88, 0:256] = emb.astype(np.float16)
    w16[88:112, 256:512] = emb.astype(np.float16)
    w16[112, 0:512] = proj_b[0:512].astype(np.float16)

    in_maps = []
    for c in range(N_CORES):
        ju = unmask_idx[c * U_C:(c + 1) * U_C].astype(np.int64)
        idxs = np.zeros((128, 2 * (UP_PAD // 16)), np.int16)
        for pr in range(2):
            flat = np.zeros(UP_PAD, np.int64)
            flat[:U_C] = ju                      # batch pr*2   (local row offset 0)
            flat[U_C:2 * U_C] = T + ju           # batch pr*2+1 (local row offset T)
            idxs[:, pr * (UP_PAD // 16):(pr + 1) * (UP_PAD // 16)] = _wrap_idxs(flat)
        penc_u = np.zeros((UP_PAD, E - 512), np.float16)
        penc_u[:U_C] = pencb16[c * U_C:(c + 1) * U_C]
        penc_u[U_C:2 * U_C] = pencb16[c * U_C:(c + 1) * U_C]
        penc_m = np.zeros((M_PAD, E - 512), np.float16)
        penc_m[:M_C] = pencb16[U + c * M_C:U + (c + 1) * M_C]
        mtokT = np.zeros((KP, M_PAD), np.float16)
        mtokT[:, :M_C] = mtokT_full[:, c * M_C:(c + 1) * M_C]
        m = {
            "pf": pf,
            "idxs": idxs,
            "penc_u": penc_u,
            "mtokT": mtokT,
            "penc_m": penc_m,
            "w": w16,
        }
        if affine:
            m["gb"] = np.stack([gamma, beta]).astype(np.float32)
        in_maps.append(m)
    return in_maps


def kernel(**inputs):
    from concourse import bass_utils

    x = np.asarray(inputs["x"], dtype=np.float32)
    proj_w = np.asarray(inputs["proj_w"], dtype=np.float32)
    proj_b = np.asarray(inputs["proj_b"], dtype=np.float32)
    mask_token = np.asarray(inputs["mask_token"], dtype=np.float32)
    gamma = np.asarray(inputs["gamma"], dtype=np.float32)
    beta = np.asarray(inputs["beta"], dtype=np.float32)
    perm = np.asarray(inputs["perm"], dtype=np.int32)

    mask_idx = perm[:M]
    unmask_idx = perm[M:]
    affine = not (np.all(gamma == 1.0) and np.all(beta == 0.0))

    # host layout prep: im2col (fp16, K padded to 128 for 256B gather rows);
    # pad rows carry onehot(h), onehot(w), ones -- see _prepare_core_inputs
    pfc = x.reshape(B, nH, P0, nH, P0, nH, P0).transpose(0, 1, 3, 5, 2, 4, 6)
    pf = np.zeros((B, T, KP), np.float16)
    pf[:, :, :K] = np.ascontiguousarray(pfc).reshape(B, T, K)
    t_ar = np.arange(T)
    pf[:, t_ar, 64 + t_ar // (nH * nH)] = 1.0
    pf[:, t_ar, 88 + (t_ar // nH) % nH] = 1.0
    pf[:, :, 112] = 1.0

    # output-ordered D-axis pos-enc chunk (+ bias tail)
    emb = _emb1_np()
    pencb = np.empty((T, E - 512), np.float32)
    pencb[:U] = emb[unmask_idx % nH] + proj_b[512:]
    pencb[U:] = emb[mask_idx % nH] + proj_b[512:]
    pencb16 = pencb.astype(np.float16)

    nc = _get_program(affine)
    in_maps = _prepare_core_inputs(pf, pencb16, mask_token, proj_w, proj_b,
                                   mask_idx, unmask_idx, gamma, beta, affine)
    res = bass_utils.run_bass_kernel_spmd(nc, in_maps, core_ids=list(range(N_CORES)))

    out_tokens = np.empty((B, T, E), np.float32)
    for c in range(N_CORES):
        out_tokens[:, c * U_C:(c + 1) * U_C] = res.results[c]["out_u"]
        out_tokens[:, U + c * M_C:U + (c + 1) * M_C] = res.results[c]["out_m"][None]
    out = out_tokens.reshape(B, nH, nH, nH, E)
    return out, mask_idx


# revision 54
# speedup vs baseline: 2.4730x; 1.0320x over previous
"""Masked 3D patch embedding kernel for Trainium2 (8 NeuronCores, SPMD).

Reference computation (see problem):
  - im2col 4x4x4 patches of x [4,96,96,96,1] -> [4, 13824, 64]
  - unmasked tokens (perm[10368:]): gather -> @proj_w -> +proj_b +penc
  - masked tokens  (perm[:10368]): mask_token @ proj_w -> +proj_b +penc
    (batch-invariant -> computed once, replicated on host assembly)
  - concat, LayerNorm(eps=1e-3) * gamma + beta
  - returns (out [4,24,24,24,768], mask_idx [10368] int32)

Device strategy (per core, SPMD over 8 cores):
  - transposing dma_gather of this core's unmasked patch rows (the
    embedding lookup) straight into PE-stationary layout [K, tokens]
  - fp16 operands on the PE (1 cycle/row vs fp32's 4), fp32 PSUM accumulate
  - pos-enc (+bias) added in PSUM via identity-matmul accumulation
  - LayerNorm: bn_stats/bn_aggr (DVE) + Sqrt/reciprocal, normalize on ACT
    straight out of PSUM, fp32 results streamed to DRAM via HWDGE
Work split: unmasked 3456 j's -> 432/core (x4 batches); masked 10368 -> 1296/core.
"""

import sys

if "/opt/trn_rl_repo" not in sys.path:
    sys.path.insert(0, "/opt/trn_rl_repo")

import numpy as np

# ---------------- problem constants (hardcoded) ----------------
B = 4
HWD = 96
P0 = 4
E = 768
nH = 24
T = nH * nH * nH          # 13824 tokens
M = int(T * 0.75)         # 10368 masked
U = T - M                 # 3456 unmasked
K = P0 * P0 * P0          # 64 patch dim
KP = 128                  # K zero-padded for the PE / 256B gather rows
EPS = 1e-3

N_CORES = 8
U_C = U // N_CORES        # 432 unmasked output tokens per core
M_C = M // N_CORES        # 1296 masked tokens per core
M_L = 512                 # masked lead-in (streamed, gather-free, 4 tiles)
M_H = (M_C - M_L) // 2    # 392 masked tokens packed into each pair stream
NTAB = 2 * T + M_H        # 28040 gather-table rows per pair (< int16 max)
SP_PAD = 1280             # packed stream: 432+432+392=1256 valid -> 10 x 128
SP_TILES = 10


def _emb1_np(n=nH):
    """per-axis sin/cos table emb1(n) -> [n, 256] float32 (channels=256)."""
    channels = int(np.ceil(E / 6) * 2)
    if channels % 2:
        channels += 1
    inv_freq = (1.0 / np.power(10000, np.arange(0, channels, 2) / np.float32(channels))).astype(np.float32)
    s = np.arange(n, dtype=np.float32)[:, None] * inv_freq[None, :]
    return np.stack([np.sin(s), np.cos(s)], axis=-1).reshape(n, channels).astype(np.float32)


def _penc_np():
    """numpy mirror of reference _sin_pos_enc_3d -> [T, E] float32."""
    channels = int(np.ceil(E / 6) * 2)
    if channels % 2:
        channels += 1
    inv_freq = (1.0 / np.power(10000, np.arange(0, channels, 2) / np.float32(channels))).astype(np.float32)

    def emb1(n):
        s = np.arange(n, dtype=np.float32)[:, None] * inv_freq[None, :]
        return np.stack([np.sin(s), np.cos(s)], axis=-1).reshape(n, channels)

    ex = np.broadcast_to(emb1(nH)[:, None, None, :], (nH, nH, nH, channels))
    ey = np.broadcast_to(emb1(nH)[None, :, None, :], (nH, nH, nH, channels))
    ez = np.broadcast_to(emb1(nH)[None, None, :, :], (nH, nH, nH, channels))
    emb = np.concatenate([ex, ey, ez], axis=-1)[..., :E]
    return np.ascontiguousarray(emb.reshape(T, E)).astype(np.float32)


# ---------------- Bass program (built once, cached) ----------------
_PROG_CACHE = {}


def _build_program(affine: bool):
    """Build the per-core Bass/Tile program. `affine`: apply gamma/beta."""
    import concourse.bass as bass
    import concourse.bacc as bacc
    import concourse.tile as tile
    from concourse import mybir
    from concourse.masks import make_identity

    f32 = mybir.dt.float32
    f16 = mybir.dt.float16
    E4 = E - 512   # D-axis pos-enc chunk (cols 512:768)

    nc = bacc.Bacc("TRN2", target_bir_lowering=False, debug=False,
                   enable_asserts=True, num_devices=N_CORES)

    pf_d = nc.dram_tensor("pf", [2, NTAB, KP], f16, kind="ExternalInput")
    idx_d = nc.dram_tensor("idxs", [128, 2 * (SP_PAD // 16)], mybir.dt.int16,
                           kind="ExternalInput")
    penc_d = nc.dram_tensor("penc", [2 * SP_PAD, E4], f16, kind="ExternalInput")
    mtokL_d = nc.dram_tensor("mtokL", [KP, M_L], f16, kind="ExternalInput")
    pencL_d = nc.dram_tensor("pencL", [M_L, E4], f16, kind="ExternalInput")
    w_d = nc.dram_tensor("w", [KP, E], f16, kind="ExternalInput")
    if affine:
        gb_d = nc.dram_tensor("gb", [2, E], f32, kind="ExternalInput")

    outu_d = nc.dram_tensor("out_u", [B, U_C, E], f16, kind="ExternalOutput")
    outm_d = nc.dram_tensor("out_m", [M_C, E], f16, kind="ExternalOutput")

    IW = SP_PAD // 16  # idx columns per batch-pair (wrapped in 16 partitions)

    with tile.TileContext(nc) as tc:
        with (
            tc.tile_pool(name="singles", bufs=1) as singles,
            tc.tile_pool(name="stats", bufs=24) as st_pool,
            tc.tile_pool(name="ypsum", bufs=4, space="PSUM") as ypsum_pool,
        ):
            # ---- resident constants; critical-path loads on the SP ring
            # (the ACT ring is blocked early by LoadActFuncSet) ----
            idx_sb = singles.tile([128, 2 * IW], mybir.dt.int16, tag="idx")
            nc.sync.dma_start(out=idx_sb[:], in_=idx_d.ap())
            w_sb = singles.tile([KP, E], f16, tag="w")
            nc.sync.dma_start(out=w_sb[:], in_=w_d.ap())
            mtokL_sb = singles.tile([KP, M_L], f16, tag="mtokL")
            nc.sync.dma_start(out=mtokL_sb[:, 0:256], in_=mtokL_d.ap()[:, 0:256])
            pml_sb = singles.tile([128, 4, E4], f16, tag="pml")
            pencL_r = pencL_d.ap().rearrange("(g p) e -> p g e", p=128)
            nc.sync.dma_start(out=pml_sb[:, 0:2, :], in_=pencL_r[:, 0:2, :])
            nc.sync.dma_start(out=mtokL_sb[:, 256:M_L], in_=mtokL_d.ap()[:, 256:M_L])
            nc.sync.dma_start(out=pml_sb[:, 2:4, :], in_=pencL_r[:, 2:4, :])
            ident = singles.tile([128, 128], f16, tag="ident")
            make_identity(nc, ident[:])
            # warm the ACT function table immediately: the LoadActFuncSet for
            # Sqrt/Copy/Identity must not land mid-pipeline behind DMA configs
            warm = singles.tile([1, 1], f32, tag="warm")
            nc.vector.memset(warm[:], 1.0)
            nc.scalar.activation(out=warm[:], in_=warm[:],
                                 func=mybir.ActivationFunctionType.Sqrt)
            nc.scalar.activation(out=warm[:], in_=warm[:],
                                 func=mybir.ActivationFunctionType.Copy)
            nc.scalar.activation(out=warm[:], in_=warm[:],
                                 func=mybir.ActivationFunctionType.Identity)
            penc_r = penc_d.ap().rearrange("(r g p) e -> r p g e", r=2, p=128)
            pe_sb = []
            for pr in range(2):
                t = singles.tile([128, SP_TILES, E4], f16, tag=f"penc{pr}")
                nc.sync.dma_start(out=t[:, 0:4, :], in_=penc_r[pr, :, 0:4, :])
                pe_sb.append(t)
            for pr in range(2):
                nc.sync.dma_start(out=pe_sb[pr][:, 4:SP_TILES, :],
                                  in_=penc_r[pr, :, 4:SP_TILES, :])
            if affine:
                gamma_sb = singles.tile([128, E], f16, tag="gamma")
                beta_sb = singles.tile([128, E], f16, tag="beta")
                gb_ap = gb_d.ap()
                nc.gpsimd.dma_start(
                    out=gamma_sb[:],
                    in_=bass.AP(tensor=gb_ap.tensor, offset=gb_ap.offset,
                                ap=[[0, 128], [1, E]]))
                nc.gpsimd.dma_start(
                    out=beta_sb[:],
                    in_=bass.AP(tensor=gb_ap.tensor, offset=gb_ap.offset + E,
                                ap=[[0, 128], [1, E]]))

            # ---- the embedding lookup: packed stream per batch-PAIR over
            # a table [pf(b0) | pf(b1) | this core's packed mask-token slice];
            # split in halves so the lead tiles land early ----
            gathT = []
            for pr in range(2):
                gt = singles.tile([128, 1, SP_PAD], f16, tag=f"gathT{pr}")
                for lo, hi in ((0, 768), (768, SP_PAD)):
                    nc.gpsimd.dma_gather(
                        gt[:, :, lo:hi], pf_d.ap()[pr],
                        idx_sb[:, pr * IW + lo // 16:pr * IW + hi // 16],
                        num_idxs=hi - lo, num_idxs_reg=hi - lo,
                        elem_size=KP, transpose=True,
                    )
                gathT.append(gt)


            def project_ln(lhsT_ap, penc_ap, o_ap, norm_on_dve=False):
                """o = LayerNorm(lhsT.T @ W + penc) into SBUF slice o_ap."""
                y = ypsum_pool.tile([128, E], f32, tag="y", space="PSUM")
                # cols 0:512: proj + H/W pos-enc + bias all in one matmul --
                # the lhsT pad rows carry onehot(h), onehot(w), and a ones slot,
                # matched by emb tables / proj_b rows in w_sb
                nc.tensor.matmul(y[:, 0:512], lhsT=lhsT_ap, rhs=w_sb[:, 0:512],
                                 start=True, stop=True)
                nc.tensor.matmul(y[:, 512:E], lhsT=lhsT_ap, rhs=w_sb[:, 512:E],
                                 start=True, stop=False)
                nc.tensor.matmul(y[:, 512:E], lhsT=ident[:], rhs=penc_ap,
                                 start=False, stop=True)
                # LayerNorm stats over E (2 x 384: equal groups keep bn_aggr exact)
                st = st_pool.tile([128, 2, 6], f32, tag="st")
                for s in range(2):
                    nc.vector.bn_stats(out=st[:, s:s + 1, :],
                                       in_=y[:, s * 384:(s + 1) * 384])
                mv = st_pool.tile([128, 2], f32, tag="mv")
                nc.vector.bn_aggr(out=mv[:], in_=st[:])
                rv = st_pool.tile([128, 1], f32, tag="rv")
                nc.vector.tensor_scalar(out=rv[:], in0=mv[:, 1:2],
                                        scalar1=EPS, scalar2=None,
                                        op0=mybir.AluOpType.add)
                nc.vector.reciprocal(out=rv[:], in_=rv[:])
                rstd = st_pool.tile([128, 1], f32, tag="rstd")
                nc.scalar.activation(out=rstd[:], in_=rv[:],
                                     func=mybir.ActivationFunctionType.Sqrt)
                if norm_on_dve:
                    # drain-tail tiles: normalize on the otherwise-idle DVE
                    nc.vector.tensor_scalar(out=o_ap, in0=y[:],
                                            scalar1=mv[:, 0:1], scalar2=rstd[:],
                                            op0=mybir.AluOpType.subtract,
                                            op1=mybir.AluOpType.mult)
                else:
                    nm = st_pool.tile([128, 1], f32, tag="nm")
                    nc.vector.tensor_scalar(out=nm[:], in0=mv[:, 0:1],
                                            scalar1=-1.0, scalar2=None,
                                            op0=mybir.AluOpType.mult)
                    nbias = st_pool.tile([128, 1], f32, tag="nbias")
                    nc.scalar.activation(out=nbias[:], in_=rstd[:],
                                         func=mybir.ActivationFunctionType.Copy,
                                         bias=0.0, scale=nm[:])
                    nc.scalar.activation(out=o_ap, in_=y[:],
                                         func=mybir.ActivationFunctionType.Identity,
                                         bias=nbias[:], scale=rstd[:])
                if affine:
                    nc.vector.tensor_mul(out=o_ap, in0=o_ap, in1=gamma_sb[:])
                    nc.vector.tensor_add(out=o_ap, in0=o_ap, in1=beta_sb[:])

            # ---- masked lead-in first: 2 streamed tiles, zero gather
            # dependency -- fills the pipeline while the gathers run ----
            om = singles.tile([128, 4, E], f16, tag="outm")
            for g in range(4):
                project_ln(mtokL_sb[:, g * 128:(g + 1) * 128],
                           pml_sb[:, g, :], om[:, g, :])
                if g == 1:
                    nc.sync.dma_start(
                        out=outm_d.ap()[0:256, :].rearrange("(a p) e -> p a e", p=128),
                        in_=om[:, 0:2, :])
            nc.sync.dma_start(
                out=outm_d.ap()[256:M_L, :].rearrange("(a p) e -> p a e", p=128),
                in_=om[:, 2:4, :])

            # ---- packed streams: 11 tiles per pair. Row r maps to:
            # r<432: (b=2pr, j=r); r<864: (b=2pr+1, j=r-432);
            # r<1384: masked m = M_L + pr*520 + (r-864) ----
            for pr in range(2):
                b0, b1 = 2 * pr, 2 * pr + 1
                m0 = M_L + pr * M_H
                ob = singles.tile([128, SP_TILES, E], f16, tag=f"outs{pr}")
                for g in range(SP_TILES):
                    project_ln(gathT[pr][:, 0, g * 128:(g + 1) * 128],
                               pe_sb[pr][:, g, :], ob[:, g, :],
                               norm_on_dve=False)
                    if g == 1:
                        nc.sync.dma_start(
                            out=outu_d.ap()[b0, 0:256, :].rearrange(
                                "(a p) e -> p a e", p=128),
                            in_=ob[:, 0:2, :])
                    if g == 3:
                        nc.sync.dma_start(out=outu_d.ap()[b0, 256:384, :],
                                          in_=ob[:, 2, :])
                        nc.sync.dma_start(out=outu_d.ap()[b0, 384:U_C, :],
                                          in_=ob[:48, 3, :])
                        nc.sync.dma_start(out=outu_d.ap()[b1, 0:80, :],
                                          in_=ob[48:128, 3, :])
                    if g == 5:
                        nc.sync.dma_start(
                            out=outu_d.ap()[b1, 80:336, :].rearrange(
                                "(a p) e -> p a e", p=128),
                            in_=ob[:, 4:6, :])
                    if g == 6:
                        nc.sync.dma_start(out=outu_d.ap()[b1, 336:U_C, :],
                                          in_=ob[:96, 6, :])
                        nc.sync.dma_start(out=outm_d.ap()[m0:m0 + 32, :],
                                          in_=ob[96:128, 6, :])
                    if g == 8:
                        nc.sync.dma_start(
                            out=outm_d.ap()[m0 + 32:m0 + 288, :].rearrange(
                                "(a p) e -> p a e", p=128),
                            in_=ob[:, 7:9, :])
                nc.sync.dma_start(out=outm_d.ap()[m0 + 288:m0 + M_H, :],
                                  in_=ob[:104, 9, :])

    nc.compile()
    return nc


def _get_program(affine: bool):
    if affine not in _PROG_CACHE:
        _PROG_CACHE[affine] = _build_program(affine)
    return _PROG_CACHE[affine]


def _wrap_idxs(idx_flat):
    """[U_PAD] int -> [128, U_PAD//16] int16: idx i at [i % 16, i // 16],
    rows replicated to all 128 partitions (16-partition pattern x 8 cores)."""
    w = np.asarray(idx_flat, np.int16).reshape(-1, 16).T        # [16, IW]
    return np.tile(w, (8, 1))                                    # [128, IW]


def _prepare_core_inputs(pf, pencb16, mask_token, proj_w, proj_b, mask_idx,
                         unmask_idx, gamma, beta, affine):
    """Build the 8 per-core input dicts (host-side layout prep only).

    lhsT pad-row encoding (rows 64:128 of pf / mtokT):
      64:88  onehot(h)   -> w rows 64:88  = embH table in cols 0:256
      88:112 onehot(w)   -> w rows 88:112 = embW table in cols 256:512
      112    ones        -> w row 112     = proj_b[0:512] in cols 0:512
    so the cols-0:512 matmul includes the H/W pos-enc + bias directly."""
    emb = _emb1_np()                                     # [24, 256]
    mtokT_full = np.zeros((KP, M), np.float16)
    mtokT_full[:K] = np.asarray(mask_token[0], np.float32).T.astype(np.float16)
    mh, mw = mask_idx // (nH * nH), (mask_idx // nH) % nH
    mtokT_full[64 + mh, np.arange(M)] = 1.0
    mtokT_full[88 + mw, np.arange(M)] = 1.0
    mtokT_full[112, :] = 1.0
    w16 = np.zeros((KP, E), np.float16)
    w16[:K] = np.asarray(proj_w, np.float32).astype(np.float16)
    w16[64:88, 0:256] = emb.astype(np.float16)
    w16[88:112, 256:512] = emb.astype(np.float16)
    w16[112, 0:512] = proj_b[0:512].astype(np.float16)

    # mask-token rows in the same 256B row format as patch rows
    mtok_rows = mtokT_full.T.copy()                      # [M, KP]

    in_maps = []
    for c in range(N_CORES):
        ju = unmask_idx[c * U_C:(c + 1) * U_C].astype(np.int64)
        mc0 = c * M_C
        # per-pair gather tables: [pf(b0) | pf(b1) | packed mask-token slice]
        tab = np.zeros((2, NTAB, KP), np.float16)
        idxs = np.zeros((128, 2 * (SP_PAD // 16)), np.int16)
        penc = np.zeros((2 * SP_PAD, E - 512), np.float16)
        for pr in range(2):
            tab[pr, 0:2 * T] = pf[2 * pr:2 * pr + 2].reshape(2 * T, KP)
            m0 = mc0 + M_L + pr * M_H
            tab[pr, 2 * T:NTAB] = mtok_rows[m0:m0 + M_H]
            flat = np.zeros(SP_PAD, np.int64)
            flat[:U_C] = ju                      # batch pr*2
            flat[U_C:2 * U_C] = T + ju           # batch pr*2+1
            flat[2 * U_C:2 * U_C + M_H] = 2 * T + np.arange(M_H)
            idxs[:, pr * (SP_PAD // 16):(pr + 1) * (SP_PAD // 16)] = _wrap_idxs(flat)
            o = pr * SP_PAD
            penc[o:o + U_C] = pencb16[c * U_C:(c + 1) * U_C]
            penc[o + U_C:o + 2 * U_C] = pencb16[c * U_C:(c + 1) * U_C]
            penc[o + 2 * U_C:o + 2 * U_C + M_H] = pencb16[U + m0:U + m0 + M_H]
        m = {
            "pf": tab,
            "idxs": idxs,
            "penc": penc,
            "mtokL": np.ascontiguousarray(mtokT_full[:, mc0:mc0 + M_L]),
            "pencL": np.ascontiguousarray(pencb16[U + mc0:U + mc0 + M_L]),
            "w": w16,
        }
        if affine:
            m["gb"] = np.stack([gamma, beta]).astype(np.float32)
        in_maps.append(m)
    return in_maps


def kernel(**inputs):
    from concourse import bass_utils

    x = np.asarray(inputs["x"], dtype=np.float32)
    proj_w = np.asarray(inputs["proj_w"], dtype=np.float32)
    proj_b = np.asarray(inputs["proj_b"], dtype=np.float32)
    mask_token = np.asarray(inputs["mask_token"], dtype=np.float32)
    gamma = np.asarray(inputs["gamma"], dtype=np.float32)
    beta = np.asarray(inputs["beta"], dtype=np.float32)
    perm = np.asarray(inputs["perm"], dtype=np.int32)

    mask_idx = perm[:M]
    unmask_idx = perm[M:]
    affine = not (np.all(gamma == 1.0) and np.all(beta == 0.0))

    # host layout prep: im2col (fp16, K padded to 128 for 256B gather rows);
    # pad rows carry onehot(h), onehot(w), ones -- see _prepare_core_inputs
    pfc = x.reshape(B, nH, P0, nH, P0, nH, P0).transpose(0, 1, 3, 5, 2, 4, 6)
    pf = np.zeros((B, T, KP), np.float16)
    pf[:, :, :K] = np.ascontiguousarray(pfc).reshape(B, T, K)
    t_ar = np.arange(T)
    pf[:, t_ar, 64 + t_ar // (nH * nH)] = 1.0
    pf[:, t_ar, 88 + (t_ar // nH) % nH] = 1.0
    pf[:, :, 112] = 1.0

    # output-ordered D-axis pos-enc chunk (+ bias tail)
    emb = _emb1_np()
    pencb = np.empty((T, E - 512), np.float32)
    pencb[:U] = emb[unmask_idx % nH] + proj_b[512:]
    pencb[U:] = emb[mask_idx % nH] + proj_b[512:]
    pencb16 = pencb.astype(np.float16)

    nc = _get_program(affine)
    in_maps = _prepare_core_inputs(pf, pencb16, mask_token, proj_w, proj_b,
                                   mask_idx, unmask_idx, gamma, beta, affine)
    res = bass_utils.run_bass_kernel_spmd(nc, in_maps, core_ids=list(range(N_CORES)))

    out_tokens = np.empty((B, T, E), np.float32)
    for c in range(N_CORES):
        out_tokens[:, c * U_C:(c + 1) * U_C] = res.results[c]["out_u"]
        out_tokens[:, U + c * M_C:U + (c + 1) * M_C] = res.results[c]["out_m"][None]
    out = out_tokens.reshape(B, nH, nH, nH, E)
    return out, mask_idx


# revision 55
# speedup vs baseline: 2.5124x; 1.0159x over previous
"""Masked 3D patch embedding kernel for Trainium2 (8 NeuronCores, SPMD).

Reference computation (see problem):
  - im2col 4x4x4 patches of x [4,96,96,96,1] -> [4, 13824, 64]
  - unmasked tokens (perm[10368:]): gather -> @proj_w -> +proj_b +penc
  - masked tokens  (perm[:10368]): mask_token @ proj_w -> +proj_b +penc
    (batch-invariant -> computed once, replicated on host assembly)
  - concat, LayerNorm(eps=1e-3) * gamma + beta
  - returns (out [4,24,24,24,768], mask_idx [10368] int32)

Device strategy (per core, SPMD over 8 cores):
  - transposing dma_gather of this core's unmasked patch rows (the
    embedding lookup) straight into PE-stationary layout [K, tokens]
  - fp16 operands on the PE (1 cycle/row vs fp32's 4), fp32 PSUM accumulate
  - pos-enc (+bias) added in PSUM via identity-matmul accumulation
  - LayerNorm: bn_stats/bn_aggr (DVE) + Sqrt/reciprocal, normalize on ACT
    straight out of PSUM, fp32 results streamed to DRAM via HWDGE
Work split: unmasked 3456 j's -> 432/core (x4 batches); masked 10368 -> 1296/core.
"""

import sys

if "/opt/trn_rl_repo" not in sys.path:
    sys.path.insert(0, "/opt/trn_rl_repo")

import numpy as np

# ---------------- problem constants (hardcoded) ----------------
B = 4
HWD = 96
P0 = 4
E = 768
nH = 24
T = nH * nH * nH          # 13824 tokens
M = int(T * 0.75)         # 10368 masked
U = T - M                 # 3456 unmasked
K = P0 * P0 * P0          # 64 patch dim
KP = 128                  # K zero-padded for the PE / 256B gather rows
EPS = 1e-3

N_CORES = 8
U_C = U // N_CORES        # 432 unmasked output tokens per core
M_C = M // N_CORES        # 1296 masked tokens per core
M_L = 512                 # masked lead-in (streamed, gather-free, 4 tiles)
M_H = (M_C - M_L) // 2    # 392 masked tokens packed into each pair stream
NTAB = 2 * T + M_H        # 28040 gather-table rows per pair (< int16 max)
SP_PAD = 1280             # packed stream: 432+432+392=1256 valid -> 10 x 128
SP_TILES = 10


def _emb1_np(n=nH):
    """per-axis sin/cos table emb1(n) -> [n, 256] float32 (channels=256)."""
    channels = int(np.ceil(E / 6) * 2)
    if channels % 2:
        channels += 1
    inv_freq = (1.0 / np.power(10000, np.arange(0, channels, 2) / np.float32(channels))).astype(np.float32)
    s = np.arange(n, dtype=np.float32)[:, None] * inv_freq[None, :]
    return np.stack([np.sin(s), np.cos(s)], axis=-1).reshape(n, channels).astype(np.float32)


def _penc_np():
    """numpy mirror of reference _sin_pos_enc_3d -> [T, E] float32."""
    channels = int(np.ceil(E / 6) * 2)
    if channels % 2:
        channels += 1
    inv_freq = (1.0 / np.power(10000, np.arange(0, channels, 2) / np.float32(channels))).astype(np.float32)

    def emb1(n):
        s = np.arange(n, dtype=np.float32)[:, None] * inv_freq[None, :]
        return np.stack([np.sin(s), np.cos(s)], axis=-1).reshape(n, channels)

    ex = np.broadcast_to(emb1(nH)[:, None, None, :], (nH, nH, nH, channels))
    ey = np.broadcast_to(emb1(nH)[None, :, None, :], (nH, nH, nH, channels))
    ez = np.broadcast_to(emb1(nH)[None, None, :, :], (nH, nH, nH, channels))
    emb = np.concatenate([ex, ey, ez], axis=-1)[..., :E]
    return np.ascontiguousarray(emb.reshape(T, E)).astype(np.float32)


# ---------------- Bass program (built once, cached) ----------------
_PROG_CACHE = {}


def _build_program(affine: bool):
    """Build the per-core Bass/Tile program. `affine`: apply gamma/beta."""
    import concourse.bass as bass
    import concourse.bacc as bacc
    import concourse.tile as tile
    from concourse import mybir
    from concourse.masks import make_identity

    f32 = mybir.dt.float32
    f16 = mybir.dt.float16
    E4 = E - 512   # D-axis pos-enc chunk (cols 512:768)

    nc = bacc.Bacc("TRN2", target_bir_lowering=False, debug=False,
                   enable_asserts=True, num_devices=N_CORES,
                   dynamic_dma_scratch_size=65536)

    pf_d = nc.dram_tensor("pf", [2, NTAB, KP], f16, kind="ExternalInput")
    idx_d = nc.dram_tensor("idxs", [128, 2 * (SP_PAD // 16)], mybir.dt.int16,
                           kind="ExternalInput")
    penc_d = nc.dram_tensor("penc", [2 * SP_PAD, E4], f16, kind="ExternalInput")
    mtokL_d = nc.dram_tensor("mtokL", [KP, M_L], f16, kind="ExternalInput")
    pencL_d = nc.dram_tensor("pencL", [M_L, E4], f16, kind="ExternalInput")
    w_d = nc.dram_tensor("w", [KP, E], f16, kind="ExternalInput")
    if affine:
        gb_d = nc.dram_tensor("gb", [2, E], f32, kind="ExternalInput")

    outu_d = nc.dram_tensor("out_u", [B, U_C, E], f16, kind="ExternalOutput")
    outm_d = nc.dram_tensor("out_m", [M_C, E], f16, kind="ExternalOutput")

    IW = SP_PAD // 16  # idx columns per batch-pair (wrapped in 16 partitions)

    with tile.TileContext(nc) as tc:
        with (
            tc.tile_pool(name="singles", bufs=1) as singles,
            tc.tile_pool(name="stats", bufs=24) as st_pool,
            tc.tile_pool(name="ypsum", bufs=4, space="PSUM") as ypsum_pool,
        ):
            # ---- resident constants; critical-path loads on the SP ring
            # (the ACT ring is blocked early by LoadActFuncSet) ----
            # lead-in chain first: w + first mask-token chunk + its pos-enc
            w_sb = singles.tile([KP, E], f16, tag="w")
            nc.sync.dma_start(out=w_sb[:], in_=w_d.ap())
            mtokL_sb = singles.tile([KP, M_L], f16, tag="mtokL")
            nc.sync.dma_start(out=mtokL_sb[:, 0:256], in_=mtokL_d.ap()[:, 0:256])
            pml_sb = singles.tile([128, 4, E4], f16, tag="pml")
            pencL_r = pencL_d.ap().rearrange("(g p) e -> p g e", p=128)
            nc.sync.dma_start(out=pml_sb[:, 0:2, :], in_=pencL_r[:, 0:2, :])
            idx_sb = singles.tile([128, 2 * IW], mybir.dt.int16, tag="idx")
            nc.sync.dma_start(out=idx_sb[:], in_=idx_d.ap())
            nc.sync.dma_start(out=mtokL_sb[:, 256:M_L], in_=mtokL_d.ap()[:, 256:M_L])
            nc.sync.dma_start(out=pml_sb[:, 2:4, :], in_=pencL_r[:, 2:4, :])
            ident = singles.tile([128, 128], f16, tag="ident")
            make_identity(nc, ident[:])
            # warm the ACT function table immediately: the LoadActFuncSet for
            # Sqrt/Copy/Identity must not land mid-pipeline behind DMA configs
            warm = singles.tile([1, 1], f32, tag="warm")
            nc.vector.memset(warm[:], 1.0)
            nc.scalar.activation(out=warm[:], in_=warm[:],
                                 func=mybir.ActivationFunctionType.Sqrt)
            nc.scalar.activation(out=warm[:], in_=warm[:],
                                 func=mybir.ActivationFunctionType.Copy)
            nc.scalar.activation(out=warm[:], in_=warm[:],
                                 func=mybir.ActivationFunctionType.Identity)
            penc_r = penc_d.ap().rearrange("(r g p) e -> r p g e", r=2, p=128)
            pe_sb = []
            for pr in range(2):
                t = singles.tile([128, SP_TILES, E4], f16, tag=f"penc{pr}")
                nc.sync.dma_start(out=t[:, 0:4, :], in_=penc_r[pr, :, 0:4, :])
                pe_sb.append(t)
            for pr in range(2):
                nc.sync.dma_start(out=pe_sb[pr][:, 4:SP_TILES, :],
                                  in_=penc_r[pr, :, 4:SP_TILES, :])
            if affine:
                gamma_sb = singles.tile([128, E], f16, tag="gamma")
                beta_sb = singles.tile([128, E], f16, tag="beta")
                gb_ap = gb_d.ap()
                nc.gpsimd.dma_start(
                    out=gamma_sb[:],
                    in_=bass.AP(tensor=gb_ap.tensor, offset=gb_ap.offset,
                                ap=[[0, 128], [1, E]]))
                nc.gpsimd.dma_start(
                    out=beta_sb[:],
                    in_=bass.AP(tensor=gb_ap.tensor, offset=gb_ap.offset + E,
                                ap=[[0, 128], [1, E]]))

            # ---- the embedding lookup: packed stream per batch-PAIR over
            # a table [pf(b0) | pf(b1) | this core's packed mask-token slice];
            # split in halves so the lead tiles land early ----
            gathT = []
            for pr in range(2):
                gt = singles.tile([128, 1, SP_PAD], f16, tag=f"gathT{pr}")
                for lo, hi in ((0, 768), (768, SP_PAD)):
                    nc.gpsimd.dma_gather(
                        gt[:, :, lo:hi], pf_d.ap()[pr],
                        idx_sb[:, pr * IW + lo // 16:pr * IW + hi // 16],
                        num_idxs=hi - lo, num_idxs_reg=hi - lo,
                        elem_size=KP, transpose=True,
                    )
                gathT.append(gt)


            def project_ln(lhsT_ap, penc_ap, o_ap, norm_on_dve=False):
                """o = LayerNorm(lhsT.T @ W + penc) into SBUF slice o_ap."""
                y = ypsum_pool.tile([128, E], f32, tag="y", space="PSUM")
                # cols 0:512: proj + H/W pos-enc + bias all in one matmul --
                # the lhsT pad rows carry onehot(h), onehot(w), and a ones slot,
                # matched by emb tables / proj_b rows in w_sb
                nc.tensor.matmul(y[:, 0:512], lhsT=lhsT_ap, rhs=w_sb[:, 0:512],
                                 start=True, stop=True)
                nc.tensor.matmul(y[:, 512:E], lhsT=lhsT_ap, rhs=w_sb[:, 512:E],
                                 start=True, stop=False)
                nc.tensor.matmul(y[:, 512:E], lhsT=ident[:], rhs=penc_ap,
                                 start=False, stop=True)
                # LayerNorm stats over E (2 x 384: equal groups keep bn_aggr exact)
                st = st_pool.tile([128, 2, 6], f32, tag="st")
                for s in range(2):
                    nc.vector.bn_stats(out=st[:, s:s + 1, :],
                                       in_=y[:, s * 384:(s + 1) * 384])
                mv = st_pool.tile([128, 2], f32, tag="mv")
                nc.vector.bn_aggr(out=mv[:], in_=st[:])
                rv = st_pool.tile([128, 1], f32, tag="rv")
                nc.vector.tensor_scalar(out=rv[:], in0=mv[:, 1:2],
                                        scalar1=EPS, scalar2=None,
                                        op0=mybir.AluOpType.add)
                nc.vector.reciprocal(out=rv[:], in_=rv[:])
                rstd = st_pool.tile([128, 1], f32, tag="rstd")
                nc.scalar.activation(out=rstd[:], in_=rv[:],
                                     func=mybir.ActivationFunctionType.Sqrt)
                if norm_on_dve:
                    # drain-tail tiles: normalize on the otherwise-idle DVE
                    nc.vector.tensor_scalar(out=o_ap, in0=y[:],
                                            scalar1=mv[:, 0:1], scalar2=rstd[:],
                                            op0=mybir.AluOpType.subtract,
                                            op1=mybir.AluOpType.mult)
                else:
                    nm = st_pool.tile([128, 1], f32, tag="nm")
                    nc.vector.tensor_scalar(out=nm[:], in0=mv[:, 0:1],
                                            scalar1=-1.0, scalar2=None,
                                            op0=mybir.AluOpType.mult)
                    nbias = st_pool.tile([128, 1], f32, tag="nbias")
                    nc.scalar.activation(out=nbias[:], in_=rstd[:],
                                         func=mybir.ActivationFunctionType.Copy,
                                         bias=0.0, scale=nm[:])
                    nc.scalar.activation(out=o_ap, in_=y[:],
                                         func=mybir.ActivationFunctionType.Identity,
                                         bias=nbias[:], scale=rstd[:])
                if affine:
                    nc.vector.tensor_mul(out=o_ap, in0=o_ap, in1=gamma_sb[:])
                    nc.vector.tensor_add(out=o_ap, in0=o_ap, in1=beta_sb[:])

            # ---- masked lead-in first: 2 streamed tiles, zero gather
            # dependency -- fills the pipeline while the gathers run ----
            om = singles.tile([128, 4, E], f16, tag="outm")
            for g in range(4):
                project_ln(mtokL_sb[:, g * 128:(g + 1) * 128],
                           pml_sb[:, g, :], om[:, g, :])
                if g == 1:
                    nc.sync.dma_start(
                        out=outm_d.ap()[0:256, :].rearrange("(a p) e -> p a e", p=128),
                        in_=om[:, 0:2, :])
            nc.sync.dma_start(
                out=outm_d.ap()[256:M_L, :].rearrange("(a p) e -> p a e", p=128),
                in_=om[:, 2:4, :])

            # ---- packed streams: 11 tiles per pair. Row r maps to:
            # r<432: (b=2pr, j=r); r<864: (b=2pr+1, j=r-432);
            # r<1384: masked m = M_L + pr*520 + (r-864) ----
            for pr in range(2):
                b0, b1 = 2 * pr, 2 * pr + 1
                m0 = M_L + pr * M_H
                ob = singles.tile([128, SP_TILES, E], f16, tag=f"outs{pr}")
                for g in range(SP_TILES):
                    project_ln(gathT[pr][:, 0, g * 128:(g + 1) * 128],
                               pe_sb[pr][:, g, :], ob[:, g, :],
                               norm_on_dve=False)
                    if g == 1:
                        nc.sync.dma_start(
                            out=outu_d.ap()[b0, 0:256, :].rearrange(
                                "(a p) e -> p a e", p=128),
                            in_=ob[:, 0:2, :])
                    if g == 3:
                        nc.sync.dma_start(out=outu_d.ap()[b0, 256:384, :],
                                          in_=ob[:, 2, :])
                        nc.sync.dma_start(out=outu_d.ap()[b0, 384:U_C, :],
                                          in_=ob[:48, 3, :])
                        nc.sync.dma_start(out=outu_d.ap()[b1, 0:80, :],
                                          in_=ob[48:128, 3, :])
                    if g == 5:
                        nc.sync.dma_start(
                            out=outu_d.ap()[b1, 80:336, :].rearrange(
                                "(a p) e -> p a e", p=128),
                            in_=ob[:, 4:6, :])
                    if g == 6:
                        nc.sync.dma_start(out=outu_d.ap()[b1, 336:U_C, :],
                                          in_=ob[:96, 6, :])
                        nc.sync.dma_start(out=outm_d.ap()[m0:m0 + 32, :],
                                          in_=ob[96:128, 6, :])
                    if g == 8:
                        nc.sync.dma_start(
                            out=outm_d.ap()[m0 + 32:m0 + 288, :].rearrange(
                                "(a p) e -> p a e", p=128),
                            in_=ob[:, 7:9, :])
                nc.sync.dma_start(out=outm_d.ap()[m0 + 288:m0 + M_H, :],
                                  in_=ob[:104, 9, :])

    nc.compile()
    return nc


def _get_program(affine: bool):
    if affine not in _PROG_CACHE:
        _PROG_CACHE[affine] = _build_program(affine)
    return _PROG_CACHE[affine]


def _wrap_idxs(idx_flat):
    """[U_PAD] int -> [128, U_PAD//16] int16: idx i at [i % 16, i // 16],
    rows replicated to all 128 partitions (16-partition pattern x 8 cores)."""
    w = np.asarray(idx_flat, np.int16).reshape(-1, 16).T        # [16, IW]
    return np.tile(w, (8, 1))                                    # [128, IW]


def _prepare_core_inputs(pf, pencb16, mask_token, proj_w, proj_b, mask_idx,
                         unmask_idx, gamma, beta, affine):
    """Build the 8 per-core input dicts (host-side layout prep only).

    lhsT pad-row encoding (rows 64:128 of pf / mtokT):
      64:88  onehot(h)   -> w rows 64:88  = embH table in cols 0:256
      88:112 onehot(w)   -> w rows 88:112 = embW table in cols 256:512
      112    ones        -> w row 112     = proj_b[0:512] in cols 0:512
    so the cols-0:512 matmul includes the H/W pos-enc + bias directly."""
    emb = _emb1_np()                                     # [24, 256]
    mtokT_full = np.zeros((KP, M), np.float16)
    mtokT_full[:K] = np.asarray(mask_token[0], np.float32).T.astype(np.float16)
    mh, mw = mask_idx // (nH * nH), (mask_idx // nH) % nH
    mtokT_full[64 + mh, np.arange(M)] = 1.0
    mtokT_full[88 + mw, np.arange(M)] = 1.0
    mtokT_full[112, :] = 1.0
    w16 = np.zeros((KP, E), np.float16)
    w16[:K] = np.asarray(proj_w, np.float32).astype(np.float16)
    w16[64:88, 0:256] = emb.astype(np.float16)
    w16[88:112, 256:512] = emb.astype(np.float16)
    w16[112, 0:512] = proj_b[0:512].astype(np.float16)

    # mask-token rows in the same 256B row format as patch rows
    mtok_rows = mtokT_full.T.copy()                      # [M, KP]

    in_maps = []
    for c in range(N_CORES):
        ju = unmask_idx[c * U_C:(c + 1) * U_C].astype(np.int64)
        mc0 = c * M_C
        # per-pair gather tables: [pf(b0) | pf(b1) | packed mask-token slice]
        tab = np.zeros((2, NTAB, KP), np.float16)
        idxs = np.zeros((128, 2 * (SP_PAD // 16)), np.int16)
        penc = np.zeros((2 * SP_PAD, E - 512), np.float16)
        for pr in range(2):
            tab[pr, 0:2 * T] = pf[2 * pr:2 * pr + 2].reshape(2 * T, KP)
            m0 = mc0 + M_L + pr * M_H
            tab[pr, 2 * T:NTAB] = mtok_rows[m0:m0 + M_H]
            flat = np.zeros(SP_PAD, np.int64)
            flat[:U_C] = ju                      # batch pr*2
            flat[U_C:2 * U_C] = T + ju           # batch pr*2+1
            flat[2 * U_C:2 * U_C + M_H] = 2 * T + np.arange(M_H)
            idxs[:, pr * (SP_PAD // 16):(pr + 1) * (SP_PAD // 16)] = _wrap_idxs(flat)
            o = pr * SP_PAD
            penc[o:o + U_C] = pencb16[c * U_C:(c + 1) * U_C]
            penc[o + U_C:o + 2 * U_C] = pencb16[c * U_C:(c + 1) * U_C]
            penc[o + 2 * U_C:o + 2 * U_C + M_H] = pencb16[U + m0:U + m0 + M_H]
        m = {
            "pf": tab,
            "idxs": idxs,
            "penc": penc,
            "mtokL": np.ascontiguousarray(mtokT_full[:, mc0:mc0 + M_L]),
            "pencL": np.ascontiguousarray(pencb16[U + mc0:U + mc0 + M_L]),
            "w": w16,
        }
        if affine:
            m["gb"] = np.stack([gamma, beta]).astype(np.float32)
        in_maps.append(m)
    return in_maps


def kernel(**inputs):
    from concourse import bass_utils

    x = np.asarray(inputs["x"], dtype=np.float32)
    proj_w = np.asarray(inputs["proj_w"], dtype=np.float32)
    proj_b = np.asarray(inputs["proj_b"], dtype=np.float32)
    mask_token = np.asarray(inputs["mask_token"], dtype=np.float32)
    gamma = np.asarray(inputs["gamma"], dtype=np.float32)
    beta = np.asarray(inputs["beta"], dtype=np.float32)
    perm = np.asarray(inputs["perm"], dtype=np.int32)

    mask_idx = perm[:M]
    unmask_idx = perm[M:]
    affine = not (np.all(gamma == 1.0) and np.all(beta == 0.0))

    # host layout prep: im2col (fp16, K padded to 128 for 256B gather rows);
    # pad rows carry onehot(h), onehot(w), ones -- see _prepare_core_inputs
    pfc = x.reshape(B, nH, P0, nH, P0, nH, P0).transpose(0, 1, 3, 5, 2, 4, 6)
    pf = np.zeros((B, T, KP), np.float16)
    pf[:, :, :K] = np.ascontiguousarray(pfc).reshape(B, T, K)
    t_ar = np.arange(T)
    pf[:, t_ar, 64 + t_ar // (nH * nH)] = 1.0
    pf[:, t_ar, 88 + (t_ar // nH) % nH] = 1.0
    pf[:, :, 112] = 1.0

    # output-ordered D-axis pos-enc chunk (+ bias tail)
    emb = _emb1_np()
    pencb = np.empty((T, E - 512), np.float32)
    pencb[:U] = emb[unmask_idx % nH] + proj_b[512:]
    pencb[U:] = emb[mask_idx % nH] + proj_b[512:]
    pencb16 = pencb.astype(np.float16)

    nc = _get_program(affine)
    in_maps = _prepare_core_inputs(pf, pencb16, mask_token, proj_w, proj_b,
                                   mask_idx, unmask_idx, gamma, beta, affine)
    res = bass_utils.run_bass_kernel_spmd(nc, in_maps, core_ids=list(range(N_CORES)))

    out_tokens = np.empty((B, T, E), np.float32)
    for c in range(N_CORES):
        out_tokens[:, c * U_C:(c + 1) * U_C] = res.results[c]["out_u"]
        out_tokens[:, U + c * M_C:U + (c + 1) * M_C] = res.results[c]["out_m"][None]
    out = out_tokens.reshape(B, nH, nH, nH, E)
    return out, mask_idx
